# revision 2
# baseline (speedup 1.0000x reference)
"""MoE (8 experts, top-2) + shared-expert SwiGLU on 8 TRN2 NeuronCores.

Expert-parallel: core e holds expert e's weights; host routes tokens
(bit-exact jax-CPU replica of the reference gate) and gathers each
expert's tokens. Shared expert is token-sharded (512 tokens/core, full
shared weights replicated). Device math is bf16 with fp32 PSUM
accumulation, all in transposed layout (no on-device transposes).
Host applies top-k gate weights and scatters expert outputs back.
"""

import time
from contextlib import ExitStack

import numpy as np
import ml_dtypes

import concourse.tile as tile
from concourse import bacc, mybir
from concourse.bass_utils import run_bass_kernel_spmd

BF16 = ml_dtypes.bfloat16
P = 128
S = 4096
C = 1024
E = 8
TOP_K = 2
H = 2744
HP = 2816          # H padded to 22*128
NH = HP // P       # 22 hidden chunks
NCC = C // P       # 8 channel chunks
TB = 512           # tokens per block
N_CORES = 8

LAST_EXEC_NS = None
_CACHE = {}


def _routing(xf32, gate_w):
    """Replicate reference gate math bit-exactly on jax-CPU."""
    import jax
    import jax.numpy as jnp

    cpu = jax.devices("cpu")[0]
    with jax.default_device(cpu):
        xj = jnp.asarray(xf32)
        logits = xj @ jnp.asarray(gate_w).T
        rw = jax.nn.softmax(logits.astype(jnp.float32), axis=-1)
        tkw, tki = jax.lax.top_k(rw, TOP_K)
        importance = rw.mean(0)
        load = jax.nn.one_hot(tki[:, 0], E, dtype=jnp.float32).mean(0)
        aux = E * (importance * load).sum()
        tkw = tkw / tkw.sum(-1, keepdims=True)
        tki_np = np.asarray(tki)
        tkw_np = np.asarray(tkw, dtype=np.float32)
        aux_np = np.float32(aux)
    return tki_np, tkw_np, aux_np


def _emit_block(nc, pools, wts, src, scol, dst, dcol):
    """One 512-token SwiGLU block: dst[:, dcol:+TB] = swiglu(src[:, scol:+TB])."""
    xp, hp, silp, stp, psh, pse = pools
    w1t, w2t, w3t = wts
    AF = mybir.ActivationFunctionType

    xt = xp.tile([P, NCC * TB], mybir.dt.bfloat16, name="xt")
    for c in range(NCC):
        nc.sync.dma_start(
            xt[:, c * TB:(c + 1) * TB],
            src[c * P:(c + 1) * P, scol:scol + TB],
        )

    ht = hp.tile([P, NH * TB], mybir.dt.bfloat16, name="ht")
    for hc in range(NH):
        ps1 = psh.tile([P, TB], mybir.dt.float32, name="ps1", tag="ps1")
        ps2 = psh.tile([P, TB], mybir.dt.float32, name="ps2", tag="ps2")
        for c in range(NCC):
            nc.tensor.matmul(
                ps1[:],
                lhsT=w1t[:, c * HP + hc * P: c * HP + hc * P + P],
                rhs=xt[:, c * TB:(c + 1) * TB],
                start=(c == 0), stop=(c == NCC - 1),
            )
        for c in range(NCC):
            nc.tensor.matmul(
                ps2[:],
                lhsT=w2t[:, c * HP + hc * P: c * HP + hc * P + P],
                rhs=xt[:, c * TB:(c + 1) * TB],
                start=(c == 0), stop=(c == NCC - 1),
            )
        sil = silp.tile([P, TB], mybir.dt.bfloat16, name="sil")
        nc.scalar.activation(sil[:], ps1[:], AF.Silu)
        nc.vector.tensor_tensor(
            out=ht[:, hc * TB:(hc + 1) * TB],
            in0=sil[:], in1=ps2[:], op=mybir.AluOpType.mult,
        )

    for c in range(NCC):
        eop = pse.tile([P, TB], mybir.dt.float32, name="eop")
        for hc in range(NH):
            nc.tensor.matmul(
                eop[:],
                lhsT=w3t[:, hc * C + c * P: hc * C + c * P + P],
                rhs=ht[:, hc * TB:(hc + 1) * TB],
                start=(hc == 0), stop=(hc == NH - 1),
            )
        st = stp.tile([P, TB], mybir.dt.float32, name="st")
        nc.scalar.activation(st[:], eop[:], AF.Copy)
        nc.sync.dma_start(dst[c * P:(c + 1) * P, dcol:dcol + TB], st[:])


def _build_program(cap):
    nc = bacc.Bacc("TRN2", target_bir_lowering=False, debug=False,
                   num_devices=N_CORES)
    bf = mybir.dt.bfloat16
    f32 = mybir.dt.float32
    d_xgT = nc.dram_tensor("xgT", [C, cap], bf, kind="ExternalInput")
    d_xsT = nc.dram_tensor("xsT", [C, TB], bf, kind="ExternalInput")
    d_w1T = nc.dram_tensor("w1T", [C, HP], bf, kind="ExternalInput")
    d_w2T = nc.dram_tensor("w2T", [C, HP], bf, kind="ExternalInput")
    d_w3T = nc.dram_tensor("w3T", [HP, C], bf, kind="ExternalInput")
    d_s1T = nc.dram_tensor("s1T", [C, HP], bf, kind="ExternalInput")
    d_s2T = nc.dram_tensor("s2T", [C, HP], bf, kind="ExternalInput")
    d_s3T = nc.dram_tensor("s3T", [HP, C], bf, kind="ExternalInput")
    d_eoT = nc.dram_tensor("eoT", [C, cap], f32, kind="ExternalOutput")
    d_ysT = nc.dram_tensor("ysT", [C, TB], f32, kind="ExternalOutput")

    neb = cap // TB
    with tile.TileContext(nc) as tc, ExitStack() as ctx:
        xp = ctx.enter_context(tc.tile_pool(name="xp", bufs=2))
        hp = ctx.enter_context(tc.tile_pool(name="hp", bufs=1))
        silp = ctx.enter_context(tc.tile_pool(name="silp", bufs=2))
        stp = ctx.enter_context(tc.tile_pool(name="stp", bufs=4))
        psh = ctx.enter_context(tc.tile_pool(name="psh", bufs=2, space="PSUM"))
        pse = ctx.enter_context(tc.tile_pool(name="pse", bufs=3, space="PSUM"))
        pools = (xp, hp, silp, stp, psh, pse)

        def load_wset(wp, da, db, dc):
            wa = wp.tile([P, NCC * HP], mybir.dt.bfloat16, name="wa", tag="wa")
            wb = wp.tile([P, NCC * HP], mybir.dt.bfloat16, name="wb", tag="wb")
            wc = wp.tile([P, NH * C], mybir.dt.bfloat16, name="wc", tag="wc")
            for c in range(NCC):
                nc.sync.dma_start(wa[:, c * HP:(c + 1) * HP],
                                  da[c * P:(c + 1) * P, :])
                nc.sync.dma_start(wb[:, c * HP:(c + 1) * HP],
                                  db[c * P:(c + 1) * P, :])
            for hc in range(NH):
                nc.sync.dma_start(wc[:, hc * C:(hc + 1) * C],
                                  dc[hc * P:(hc + 1) * P, :])
            return wa, wb, wc

        with tc.tile_pool(name="wp0", bufs=1) as wp0:
            wts = load_wset(wp0, d_w1T, d_w2T, d_w3T)
            for b in range(neb):
                _emit_block(nc, pools, wts, d_xgT, b * TB, d_eoT, b * TB)
        with tc.tile_pool(name="wp1", bufs=1) as wp1:
            wts = load_wset(wp1, d_s1T, d_s2T, d_s3T)
            _emit_block(nc, pools, wts, d_xsT, 0, d_ysT, 0)

    nc.finalize()
    return nc


def _make_runner(nc, n_cores):
    import jax
    from concourse import bass2jax
    from jax.experimental.shard_map import shard_map
    from jax.sharding import Mesh, PartitionSpec

    bass2jax.install_neuronx_cc_hook()
    partition_name = nc.partition_id_tensor.name if nc.partition_id_tensor else None

    in_names, out_names, out_avals, zero_shapes = [], [], [], []
    for alloc in nc.m.functions[0].allocations:
        if not isinstance(alloc, mybir.MemoryLocationSet):
            continue
        name = alloc.memorylocations[0].name
        if alloc.kind == "ExternalInput":
            if name != partition_name:
                in_names.append(name)
        elif alloc.kind == "ExternalOutput":
            out_names.append(name)
            shape = tuple(alloc.tensor_shape)
            dtype = mybir.dt.np(alloc.dtype)
            out_avals.append(jax.core.ShapedArray(shape, dtype))
            zero_shapes.append((shape, dtype))
    n_params = len(in_names)
    n_outs = len(out_names)
    all_names = list(in_names) + list(out_names)
    if partition_name is not None:
        all_names.append(partition_name)
    all_names = tuple(all_names)
    donate = tuple(range(n_params, n_params + n_outs))

    def _body(*args):
        operands = list(args)
        if partition_name is not None:
            operands.append(bass2jax.partition_id_tensor())
        outs = bass2jax._bass_exec_p.bind(
            *operands,
            out_avals=tuple(out_avals),
            in_names=all_names,
            out_names=tuple(out_names),
            lowering_input_output_aliases=(),
            sim_require_finite=True,
            sim_require_nnan=True,
            nc=nc,
        )
        return tuple(outs)

    devices = jax.devices()[:n_cores]
    mesh = Mesh(np.asarray(devices), ("core",))
    in_specs = (PartitionSpec("core"),) * (n_params + n_outs)
    out_specs = (PartitionSpec("core"),) * n_outs
    sharded = jax.jit(
        shard_map(_body, mesh=mesh, in_specs=in_specs,
                  out_specs=out_specs, check_rep=False),
        donate_argnums=donate, keep_unused=True,
    )
    return sharded, in_names, out_names, out_avals, zero_shapes, mesh


def _run(nc, in_maps, cap, timing_reps=3):
    global LAST_EXEC_NS
    import jax
    from jax.sharding import NamedSharding, PartitionSpec

    key = ("runner", cap)
    if key not in _CACHE:
        _CACHE[key] = _make_runner(nc, N_CORES)
    sharded, in_names, out_names, out_avals, zero_shapes, mesh = _CACHE[key]

    sh = NamedSharding(mesh, PartitionSpec("core"))
    concat_in = [
        np.concatenate([np.asarray(in_maps[c][nm]) for c in range(N_CORES)], axis=0)
        for nm in in_names
    ]
    dev_in = [jax.device_put(a, sh) for a in concat_in]

    def zero_set():
        return [
            jax.device_put(np.zeros((N_CORES * s[0], *s[1:]), dt), sh)
            for s, dt in zero_shapes
        ]

    zsets = [zero_set() for _ in range(timing_reps + 1)]
    jax.block_until_ready((dev_in, zsets))

    out_arrs = sharded(*dev_in, *zsets[0])
    jax.block_until_ready(out_arrs)
    results = [
        {name: np.asarray(out_arrs[i]).reshape(N_CORES, *out_avals[i].shape)[c]
         for i, name in enumerate(out_names)}
        for c in range(N_CORES)
    ]

    times = []
    for zs in zsets[1:]:
        t0 = time.perf_counter()
        o = sharded(*dev_in, *zs)
        jax.block_until_ready(o)
        times.append(time.perf_counter() - t0)
    if times:
        LAST_EXEC_NS = int(min(times) * 1e9)
    return results


def kernel(x, gate_w, w1, w2, w3, sw1, sw2, sw3):
    x = np.asarray(x, dtype=np.float32)
    gate_w = np.asarray(gate_w, dtype=np.float32)
    xf32 = np.ascontiguousarray(x.reshape(S, C))

    tki, tkw, aux = _routing(xf32, gate_w)

    sel = np.zeros((S, E), dtype=bool)
    for k in range(TOP_K):
        sel[np.arange(S), tki[:, k]] = True
    counts = sel.sum(0)
    toks = [np.nonzero(sel[:, e])[0] for e in range(E)]
    cap = max(TB, int(-(-counts.max() // TB)) * TB)

    x_bf = xf32.astype(BF16)
    sw1T = np.zeros((C, HP), BF16)
    sw1T[:, :H] = np.asarray(sw1, np.float32).astype(BF16).T
    sw2T = np.zeros((C, HP), BF16)
    sw2T[:, :H] = np.asarray(sw2, np.float32).astype(BF16).T
    sw3T = np.zeros((HP, C), BF16)
    sw3T[:H, :] = np.asarray(sw3, np.float32).astype(BF16).T

    in_maps = []
    for e in range(E):
        xgT = np.zeros((C, cap), BF16)
        n = counts[e]
        if n:
            xgT[:, :n] = x_bf[toks[e]].T
        xsT = np.ascontiguousarray(x_bf[TB * e:TB * (e + 1)].T)
        w1T = np.zeros((C, HP), BF16)
        w1T[:, :H] = np.asarray(w1[e], np.float32).astype(BF16).T
        w2T = np.zeros((C, HP), BF16)
        w2T[:, :H] = np.asarray(w2[e], np.float32).astype(BF16).T
        w3T = np.zeros((HP, C), BF16)
        w3T[:H, :] = np.asarray(w3[e], np.float32).astype(BF16).T
        in_maps.append({
            "xgT": xgT, "xsT": xsT,
            "w1T": w1T, "w2T": w2T, "w3T": w3T,
            "s1T": sw1T, "s2T": sw2T, "s3T": sw3T,
        })

    key = ("nc", cap)
    if key not in _CACHE:
        _CACHE[key] = _build_program(cap)
    nc = _CACHE[key]

    results = _run(nc, in_maps, cap)

    y = np.empty((S, C), dtype=np.float32)
    for e in range(E):
        y[TB * e:TB * (e + 1)] = results[e]["ysT"].T
    for e in range(E):
        t = toks[e]
        if len(t) == 0:
            continue
        ke = np.where(tki[t, 0] == e, 0, 1)
        wv = tkw[t, ke].astype(np.float32)
        y[t] += results[e]["eoT"][:, :counts[e]].T * wv[:, None]

    return y.reshape(4, 1024, C), aux
